# revision 3
# baseline (speedup 1.0000x reference)
"""Trainium2 Bass kernel for LowRankRayTracer.

csi[f] = (delta_t/D) * v_f^T M v_f,  M = conj(rad)^T conj(att)  (R=32, complex)
contracted over N = D*K = 524288 rows.

Strategy (8 cores):
  - Shard the N rows across cores (512 directions each). csi is linear in M,
    so each core computes its partial S = rad32^T att32 (64x64, f32 view of
    complex pairs -> all four real cross products at once), builds
    W = [W_real | W_imag] (block form), computes partial csi over ALL F=8192
    subcarriers, and the host just sums the 8 partial csi vectors.
  - Main loop: stream 16+16 MiB per core as (128, 8192) f32 tiles; 64 stacked
    matmuls per tile pair accumulate into one PSUM (128,128) bank (2 row-chunks
    stacked -> diagonal blocks summed later via selection matmuls).
"""

import numpy as np

D, K, R = 4096, 128, 32
F = 8192
N_CORES = 8
DIR_PER_CORE = D // N_CORES              # 512
ROWS_PER_CORE = DIR_PER_CORE * K         # 65536 rows of (64,) f32
N_MACRO = 4                              # macro tiles per tensor per core
MACRO_COLS = 8192                        # f32 per partition per macro tile
SLICE = 128                              # matmul slice width (2 rows/partition)
SCALE = (200.0 / K) / D                  # delta_t / num_directions (exact binary)
FCHUNK = 512                             # phase-3 subcarriers per chunk
N_FCHUNK = F // FCHUNK                   # 16

_NC_CACHE = {}


def _build_consts():
    """(128, 258) f32: four (128,64) selection matrices + ones-selector cols."""
    c = np.zeros((128, 258), np.float32)
    EA = np.zeros((128, 32), np.float32)
    OA = np.zeros((128, 32), np.float32)
    EB = np.zeros((128, 32), np.float32)
    OB = np.zeros((128, 32), np.float32)
    for m in range(32):
        EA[2 * m, m] = 1.0
        OA[2 * m + 1, m] = 1.0
        EB[64 + 2 * m, m] = 1.0
        OB[64 + 2 * m + 1, m] = 1.0
    c[:, 0:32] = EA
    c[:, 32:64] = OA
    c[:, 64:96] = EB
    c[:, 96:128] = OB
    c[:, 128:160] = OA
    c[:, 160:192] = EA
    c[:, 192:224] = OB
    c[:, 224:256] = EB
    c[0:64, 256] = 1.0
    c[64:128, 257] = 1.0
    return c


def build_nc(n_macro=N_MACRO):
    import concourse.bacc as bacc
    import concourse.mybir as mybir
    import concourse.tile as tile

    fp32 = mybir.dt.float32
    nc = bacc.Bacc(trn_type="TRN2", target_bir_lowering=False, debug=False)

    rad_d = nc.dram_tensor("rad", [n_macro, 128, MACRO_COLS], fp32,
                           kind="ExternalInput").ap()
    att_d = nc.dram_tensor("att", [n_macro, 128, MACRO_COLS], fp32,
                           kind="ExternalInput").ap()
    gtd_d = nc.dram_tensor("gtd", [128, F], fp32, kind="ExternalInput").ap()
    cst_d = nc.dram_tensor("consts", [128, 258], fp32, kind="ExternalInput").ap()
    out_d = nc.dram_tensor("csi", [2, F], fp32, kind="ExternalOutput").ap()

    with tile.TileContext(nc) as tc:
        with (
            tc.tile_pool(name="io", bufs=2) as io_pool,
            tc.tile_pool(name="small", bufs=1) as small,
            tc.tile_pool(name="epool", bufs=2) as epool,
            tc.tile_pool(name="spsum", bufs=1, space="PSUM") as spsum,
            tc.tile_pool(name="vpsum", bufs=1, space="PSUM") as vpsum,
            tc.tile_pool(name="tpsum", bufs=2, space="PSUM") as tpsum,
            tc.tile_pool(name="cpsum", bufs=2, space="PSUM") as cpsum,
        ):
            # constants / frequency basis (off the main DMA ring)
            c_sb = small.tile([128, 258], fp32, tag="consts")
            nc.scalar.dma_start(c_sb[:], cst_d[:])
            gtd_sb = small.tile([128, F], fp32, tag="gtd")
            nc.scalar.dma_start(gtd_sb[:], gtd_d[:])

            # ---- main loop: S128 += rad_slice^T @ att_slice ----
            s128 = spsum.tile([128, 128], fp32, tag="s128")
            n_slices = MACRO_COLS // SLICE
            total = n_macro * n_slices
            idx = 0
            for i in range(n_macro):
                rad_t = io_pool.tile([128, MACRO_COLS], fp32, tag="rad")
                nc.sync.dma_start(rad_t[:], rad_d[i, :, :])
                att_t = io_pool.tile([128, MACRO_COLS], fp32, tag="att")
                nc.sync.dma_start(att_t[:], att_d[i, :, :])
                for s in range(n_slices):
                    sl = slice(s * SLICE, (s + 1) * SLICE)
                    nc.tensor.matmul(
                        s128[:],
                        lhsT=rad_t[:, sl],
                        rhs=att_t[:, sl],
                        start=(idx == 0),
                        stop=(idx == total - 1),
                    )
                    idx += 1

            # ---- epilogue: build W = [W_real | W_imag] (64, 128) ----
            s_sb = small.tile([128, 128], fp32, tag="s_sb")
            nc.vector.tensor_copy(s_sb[:], s128[:])

            v1 = vpsum.tile([64, 64], fp32, tag="v1")
            nc.tensor.matmul(v1[:], lhsT=c_sb[:, 0:64], rhs=s_sb[:, 0:64],
                             start=True, stop=False)
            nc.tensor.matmul(v1[:], lhsT=c_sb[:, 64:128], rhs=s_sb[:, 64:128],
                             start=False, stop=True)
            v2 = vpsum.tile([64, 64], fp32, tag="v2")
            nc.tensor.matmul(v2[:], lhsT=c_sb[:, 128:192], rhs=s_sb[:, 0:64],
                             start=True, stop=False)
            nc.tensor.matmul(v2[:], lhsT=c_sb[:, 192:256], rhs=s_sb[:, 64:128],
                             start=False, stop=True)

            v1s = small.tile([64, 64], fp32, tag="v1s")
            nc.vector.tensor_copy(v1s[:], v1[:])
            v2s = small.tile([64, 64], fp32, tag="v2s")
            nc.vector.tensor_copy(v2s[:], v2[:])

            # mr = Mr (dup-stacked), mp = -Mi (dup-stacked)
            mr = small.tile([64, 32], fp32, tag="mr")
            mp = small.tile([64, 32], fp32, tag="mp")
            nc.vector.tensor_sub(mr[0:32, :], v1s[0:32, 0:64:2], v2s[0:32, 1:64:2])
            nc.vector.tensor_sub(mr[32:64, :], v2s[32:64, 0:64:2], v1s[32:64, 1:64:2])
            nc.vector.tensor_add(mp[0:32, :], v1s[0:32, 1:64:2], v2s[0:32, 0:64:2])
            nc.vector.tensor_add(mp[32:64, :], v2s[32:64, 1:64:2], v1s[32:64, 0:64:2])

            wri = small.tile([64, 128], fp32, tag="wri")
            s_ = float(SCALE)
            # W_real = [[Mr, -Mi], [-Mi, -Mr]] * s
            nc.scalar.mul(wri[0:32, 0:32], mr[0:32, :], s_)
            nc.scalar.mul(wri[0:32, 32:64], mp[0:32, :], s_)
            nc.scalar.mul(wri[32:64, 0:32], mp[32:64, :], s_)
            nc.scalar.mul(wri[32:64, 32:64], mr[32:64, :], -s_)
            # W_imag = [[Mi, Mr], [Mr, -Mi]] * s
            nc.scalar.mul(wri[0:32, 64:96], mp[0:32, :], -s_)
            nc.scalar.mul(wri[0:32, 96:128], mr[0:32, :], s_)
            nc.scalar.mul(wri[32:64, 64:96], mr[32:64, :], s_)
            nc.scalar.mul(wri[32:64, 96:128], mp[32:64, :], s_)

            # ---- phase 3: csi chunks over F ----
            csi_sb = small.tile([2, F], fp32, tag="csi_sb")
            for ci in range(N_FCHUNK):
                fs = slice(ci * FCHUNK, (ci + 1) * FCHUNK)
                t_ps = tpsum.tile([128, FCHUNK], fp32, tag="t")
                nc.tensor.matmul(t_ps[:], lhsT=wri[:], rhs=gtd_sb[0:64, fs],
                                 start=True, stop=True)
                e_sb = epool.tile([128, FCHUNK], fp32, tag="e")
                nc.vector.tensor_mul(e_sb[:], gtd_sb[:, fs], t_ps[:])
                c_ps = cpsum.tile([2, FCHUNK], fp32, tag="c")
                nc.tensor.matmul(c_ps[:], lhsT=c_sb[:, 256:258], rhs=e_sb[:],
                                 start=True, stop=True)
                nc.vector.tensor_copy(csi_sb[:, fs], c_ps[:])

            nc.sync.dma_start(out_d[:], csi_sb[:])

    nc.compile()
    return nc


def _prep_shared(fbv):
    """gtd (128, F) f32: [Fr^T; Fi^T] duplicated, from complex fbv (F, R)."""
    fbv32 = np.ascontiguousarray(fbv).view(np.float32).reshape(F, 2 * R)
    gbt = np.concatenate([fbv32[:, 0::2].T, fbv32[:, 1::2].T], axis=0)
    return np.ascontiguousarray(np.concatenate([gbt, gbt], axis=0))


def _shard(arr, core):
    """(D/8, K, R) complex64 slice -> (N_MACRO, 128, MACRO_COLS) f32 view."""
    sh = arr[core * DIR_PER_CORE:(core + 1) * DIR_PER_CORE]
    return np.ascontiguousarray(sh).view(np.float32).reshape(
        N_MACRO, 128, MACRO_COLS)


def kernel(attenuation_vectors, radiation_vectors, frequency_basis_vectors):
    from concourse.bass_utils import run_bass_kernel_spmd

    if "nc" not in _NC_CACHE:
        _NC_CACHE["nc"] = build_nc()
    nc = _NC_CACHE["nc"]

    gtd = _prep_shared(frequency_basis_vectors)
    consts = _build_consts()
    in_maps = []
    for c in range(N_CORES):
        in_maps.append({
            "rad": _shard(radiation_vectors, c),
            "att": _shard(attenuation_vectors, c),
            "gtd": gtd,
            "consts": consts,
        })

    res = run_bass_kernel_spmd(nc, in_maps, core_ids=list(range(N_CORES)))
    acc = np.zeros((2, F), np.float64)
    for r in res.results:
        acc += r["csi"]
    return (acc[0] + 1j * acc[1]).astype(np.complex64)
